# revision 51
# baseline (speedup 1.0000x reference)
"""Block-diagonal linear kernel for 8 trn2 NeuronCores.

Problem: out = block_diag(blocks) @ inp + bias[:, None]
  inp:    (2048, 8192) f32   (= 8 blocks x 256 rows, 8192 token columns)
  blocks: (8, 256, 256) f32
  bias:   (2048,) f32
  out:    (2048, 8192) f32

Sharding: block/row parallel — core c owns block c, i.e. rows
[c*256, (c+1)*256) of inp and out, plus blocks[c] and bias slice.
Per-core traffic: 8MB in + 8MB out (DMA-bound at ~360GB/s/core).

Per-core bass kernel: x (256, 8192) is 2 contraction k-tiles of 128
partitions; out rows are 2 m-tiles of 128. For each 2048-wide n-chunk:
DMA both x k-tiles in, run 2(m) x 2(k) x 4(n512) float16 matmuls
accumulating in fp32 PSUM (fp16: 10-bit mantissa, 1 cycle/row, half the
input bytes of f32; host casts x/wt to fp16 -> rel err 2.9e-4), add bias
during the PSUM->SBUF drain split across DVE (tensor_scalar_add) and ACT
(Identity activation with per-partition bias), DMA out. ~20-32us device
time (4MB in + 8MB out per core; per-core DMA aggregate ~620GB/s).
Variant ladder (BD_MM_DTYPE): f32 exact (3e-7) but 4cyc/row, ~55-70us;
f32r (~11-bit round, 1.47e-4) ~27-39us; bf16 (2.4e-3) ~24us; fp16
default dominates: passes every gate f32r passes, ~15% faster. fp16
range is safe for randn-scale data (max |x| << 65504); subnormal tail
(<6e-5) is absorbed in the measured 2.9e-4.
"""

import os

import numpy as np

NUM_BLOCKS = 8
BLOCK_DIM = 256
N_ROWS = NUM_BLOCKS * BLOCK_DIM  # 2048
B_COLS = 8192
N_CORES = 8
P = 128

# Tunables (hardcoded defaults are the shipped config; env vars only for dev)
MM_DTYPE = os.environ.get("BD_MM_DTYPE", "fp16")  # f32 | f32r | bf16 | fp16
NCHUNK = int(os.environ.get("BD_NCHUNK", "2048"))
NSPLIT = 512  # moving-operand width per matmul (fp32 max, = 1 PSUM bank)
REPS = int(os.environ.get("BD_REPS", "1"))  # timing-only: repeat body in-NEFF
MODE = os.environ.get("BD_MODE", "normal")  # normal | copy | read | write (probes)
BIAS_MODE = os.environ.get("BD_BIAS", "dve_act")  # dve | dve_act | pe
DRAIN = os.environ.get("BD_DRAIN", "dve")  # dve | act | split (only BIAS=pe)
XBUFS = int(os.environ.get("BD_XBUFS", "4"))
OBUFS = int(os.environ.get("BD_OBUFS", "3"))
DMA_IN = os.environ.get("BD_DMA_IN", "sync")  # sync (HWDGE) | gpsimd (SWDGE)
PREFETCH = int(os.environ.get("BD_PREFETCH", "0"))  # issue in-DMA n+1 before out n
WLATE = int(os.environ.get("BD_WLATE", "0"))  # weights after first x chunk
OSPLIT = int(os.environ.get("BD_OSPLIT", "0"))  # split out-DMA into halves
MERGEK = int(os.environ.get("BD_MERGEK", "0"))  # one 2MB in-DMA for both k-tiles

_RUNNER = None  # cached (jitted callable, metadata)


def _install_neff_cache():
    """Memoize concourse's walrus compile (bir json -> NEFF) on disk.

    The bass_exec jit path recompiles the NEFF (~1-2 min of walrus) in every
    fresh process because it bypasses the standard neuronx-cc cache. The bir
    json is deterministic for this kernel, so a content-keyed NEFF cache makes
    repeat process startups take seconds. Fail-open: any error falls back to
    the original compile path.
    """
    try:
        import hashlib
        import shutil
        from pathlib import Path

        import concourse.bass2jax as b2j

        if getattr(b2j, "_bd_neff_cache_installed", False):
            return
        orig = b2j.compile_bir_kernel
        cache_dir = Path(os.environ.get("BD_NEFF_CACHE", "/root/.cache/bd_neff"))

        def cached_compile(bir_json, tmpdir, neff_name="file.neff"):
            try:
                raw = bir_json if isinstance(bir_json, bytes) else bir_json.encode()
                key = hashlib.sha256(raw + neff_name.encode()).hexdigest()
                cpath = cache_dir / f"{key}.neff"
                if cpath.exists():
                    out = Path(tmpdir) / neff_name
                    shutil.copyfile(cpath, out)
                    return str(out)
                neff_file = orig(bir_json, tmpdir, neff_name=neff_name)
                cache_dir.mkdir(parents=True, exist_ok=True)
                tmp = cache_dir / f".{key}.{os.getpid()}.tmp"
                shutil.copyfile(neff_file, tmp)
                tmp.rename(cpath)
                return neff_file
            except Exception:
                return orig(bir_json, tmpdir, neff_name=neff_name)

        b2j.compile_bir_kernel = cached_compile
        b2j._bd_neff_cache_installed = True
    except Exception:
        pass


def import_act_identity():
    import concourse.mybir as mybir

    return mybir.ActivationFunctionType.Identity


def _build_nc():
    import concourse.mybir as mybir
    from concourse import bacc
    from concourse.tile import TileContext

    f32 = mybir.dt.float32
    mm_dt = {
        "f32": mybir.dt.float32,
        "f32r": mybir.dt.float32r,
        "bf16": mybir.dt.bfloat16,
        "fp16": mybir.dt.float16,
    }[MM_DTYPE]

    nc = bacc.Bacc(
        "TRN2",
        target_bir_lowering=False,
        debug=False,
        enable_asserts=False,
        num_devices=N_CORES,
    )

    x_d = nc.dram_tensor("x", (BLOCK_DIM, B_COLS), mm_dt, kind="ExternalInput")
    wt_d = nc.dram_tensor("wt", (BLOCK_DIM, BLOCK_DIM), mm_dt, kind="ExternalInput")
    b_d = nc.dram_tensor("b", (P, BLOCK_DIM // P), f32, kind="ExternalInput")
    out_d = nc.dram_tensor("out", (BLOCK_DIM, B_COLS), f32, kind="ExternalOutput")

    x_ap = x_d.ap()
    wt_ap = wt_d.ap()
    b_ap = b_d.ap()
    out_ap = out_d.ap()

    KT = BLOCK_DIM // P  # 2 contraction k-tiles
    MT = BLOCK_DIM // P  # 2 output m-tiles
    NCH = B_COLS // NCHUNK  # n-chunks
    NS = NCHUNK // NSPLIT  # matmuls per psum tile per k

    with TileContext(nc) as tc:
        with (
            tc.tile_pool(name="const", bufs=1) as const,
            tc.tile_pool(name="xp", bufs=XBUFS) as xp,
            tc.tile_pool(name="op", bufs=OBUFS) as op,
            tc.tile_pool(name="psp", bufs=2, space="PSUM") as psp,
        ):
            def load_consts():
                wt_tiles = []
                for kt in range(KT):
                    wtile = const.tile(
                        [P, BLOCK_DIM], mm_dt, tag=f"wt{kt}", name=f"wt{kt}"
                    )
                    nc.sync.dma_start(out=wtile, in_=wt_ap[kt * P : (kt + 1) * P, :])
                    wt_tiles.append(wtile)
                bias_tile = const.tile(
                    [P, BLOCK_DIM // P], f32, tag="bias", name="bias"
                )
                nc.sync.dma_start(out=bias_tile, in_=b_ap)
                return wt_tiles, bias_tile

            if not WLATE:
                wt_tiles, bias_tile = load_consts()
            if BIAS_MODE == "pe":
                # bias as a K=1 matmul: lhsT [1, 256] bias row, rhs [1, N] ones
                # bias_rows[mt][0, p] = bias[mt*128+p]: K=1 stationary operand
                # for the bias matmul of m-tile mt (base_partition must be 0).
                b_t = b_ap.rearrange("p m -> m p").bitcast(mm_dt)
                bias_rows = []
                for mt in range(MT):
                    br = const.tile([1, P], mm_dt, tag=f"biasrow{mt}", name=f"biasrow{mt}")
                    nc.sync.dma_start(out=br, in_=b_t[mt : mt + 1, :])
                    bias_rows.append(br)
                ones_f32 = const.tile([1, NSPLIT], f32, tag="ones32", name="ones_f32")
                nc.vector.memset(ones_f32, 1.0)
                ones_row = const.tile([1, NSPLIT], mm_dt, tag="ones", name="ones_row")
                nc.vector.tensor_copy(out=ones_row, in_=ones_f32)

            if MODE == "write":
                wsrc_tile = const.tile([P, NCHUNK], mm_dt, tag="wsrc", name="wsrc")
                nc.sync.dma_start(out=wsrc_tile, in_=x_ap[0:P, 0:NCHUNK])

            if os.environ.get("BD_RAMP", "1") == "1" or getattr(_build_nc, "_ramp", False):
                sizes = [512, 512, 1024] + [NCHUNK] * ((B_COLS - 2048) // NCHUNK)
            else:
                sizes = [NCHUNK] * NCH
            starts = np.cumsum([0] + sizes[:-1]).tolist()
            assert sum(sizes) == B_COLS

            x_kpn = x_ap.rearrange("(kt p) n -> p kt n", p=P)

            def load_chunk(rep, n, n0, nsz):
                dma_eng = nc.gpsimd if DMA_IN == "gpsimd" else nc.sync
                if MERGEK:
                    xm = xp.tile(
                        [P, KT, nsz], mm_dt, tag="xm", name=f"xm_{rep}_{n}"
                    )
                    dma_eng.dma_start(out=xm, in_=x_kpn[:, :, n0 : n0 + nsz])
                    return [xm[:, kt, :] for kt in range(KT)]
                xts = []
                for kt in range(KT):
                    xt = xp.tile(
                        [P, nsz], mm_dt, tag=f"x{kt}", name=f"x{kt}_{rep}_{n}"
                    )
                    dma_eng.dma_start(
                        out=xt, in_=x_ap[kt * P : (kt + 1) * P, n0 : n0 + nsz]
                    )
                    xts.append(xt)
                return xts

            prefetched = None
            if PREFETCH:
                prefetched = load_chunk(0, 0, starts[0], sizes[0])
                if WLATE:
                    wt_tiles, bias_tile = load_consts()
            elif WLATE:
                wt_tiles, bias_tile = load_consts()

            for rep in range(REPS):
                for n, (n0, nsz) in enumerate(zip(starts, sizes)):
                    ncols = slice(n0, n0 + nsz)
                    nsp = nsz // NSPLIT
                    if MODE == "write":
                        # write-BW probe: store a preloaded tile repeatedly
                        for mt in range(MT):
                            nc.sync.dma_start(
                                out=out_ap[mt * P : (mt + 1) * P, ncols].bitcast(
                                    mm_dt
                                ),
                                in_=wsrc_tile[:, :nsz],
                            )
                        continue
                    if PREFETCH:
                        xts = prefetched
                        if n + 1 < len(starts):
                            prefetched = load_chunk(rep, n + 1, starts[n + 1], sizes[n + 1])
                        elif rep + 1 < REPS:
                            prefetched = load_chunk(rep + 1, 0, starts[0], sizes[0])
                    else:
                        xts = load_chunk(rep, n, n0, nsz)
                    if MODE == "copy":
                        # DMA-floor probe: store the loaded tiles straight back
                        for kt in range(KT):
                            nc.sync.dma_start(
                                out=out_ap[kt * P : (kt + 1) * P, ncols].bitcast(
                                    mm_dt
                                ),
                                in_=xts[kt],
                            )
                        continue
                    if MODE == "read":
                        # read-BW probe: tiny dependent store so loads survive DCE
                        for kt in range(KT):
                            nc.sync.dma_start(
                                out=out_ap[
                                    kt * P : (kt + 1) * P, n0 + rep : n0 + rep + 1
                                ].bitcast(mm_dt),
                                in_=xts[kt][:, 0:1],
                            )
                        continue
                    for mt in range(MT):
                        ps = psp.tile([P, nsz], f32, tag="ps", name=f"ps{rep}_{n}_{mt}")
                        for kt in range(KT):
                            lhsT = wt_tiles[kt][:, mt * P : (mt + 1) * P]
                            for j in range(nsp):
                                nc.tensor.matmul(
                                    ps[:, j * NSPLIT : (j + 1) * NSPLIT],
                                    lhsT,
                                    xts[kt][:, j * NSPLIT : (j + 1) * NSPLIT],
                                    start=(kt == 0),
                                    stop=(BIAS_MODE != "pe" and kt == KT - 1),
                                )
                        if BIAS_MODE == "pe":
                            for j in range(nsp):
                                nc.tensor.matmul(
                                    ps[:, j * NSPLIT : (j + 1) * NSPLIT],
                                    bias_rows[mt],
                                    ones_row,
                                    start=False,
                                    stop=True,
                                )
                        if BIAS_MODE == "pe" and DRAIN == "none":
                            # DMA straight from PSUM to DRAM; bias already in
                            nc.sync.dma_start(
                                out=out_ap[mt * P : (mt + 1) * P, ncols], in_=ps
                            )
                            continue
                        ot = op.tile([P, nsz], f32, tag="o", name=f"o{rep}_{n}_{mt}")
                        if BIAS_MODE == "dve":
                            nc.vector.tensor_scalar_add(
                                out=ot, in0=ps, scalar1=bias_tile[:, mt : mt + 1]
                            )
                        elif BIAS_MODE == "dve_act":
                            half = nsz // 2
                            nc.vector.tensor_scalar_add(
                                out=ot[:, :half],
                                in0=ps[:, :half],
                                scalar1=bias_tile[:, mt : mt + 1],
                            )
                            nc.scalar.activation(
                                ot[:, half:],
                                ps[:, half:],
                                import_act_identity(),
                                bias=bias_tile[:, mt : mt + 1],
                                scale=1.0,
                            )
                        else:
                            half = nsz // 2
                            if DRAIN == "dve":
                                nc.vector.tensor_copy(out=ot, in_=ps)
                            elif DRAIN == "act":
                                nc.scalar.copy(ot, ps)
                            else:  # split
                                nc.vector.tensor_copy(
                                    out=ot[:, :half], in_=ps[:, :half]
                                )
                                nc.scalar.copy(ot[:, half:], ps[:, half:])
                        if OSPLIT and nsz >= 1024:
                            h = nsz // 2
                            nc.sync.dma_start(
                                out=out_ap[mt * P : (mt + 1) * P, n0 : n0 + h],
                                in_=ot[:, :h],
                            )
                            nc.sync.dma_start(
                                out=out_ap[mt * P : (mt + 1) * P, n0 + h : n0 + nsz],
                                in_=ot[:, h:],
                            )
                        else:
                            nc.sync.dma_start(
                                out=out_ap[mt * P : (mt + 1) * P, ncols], in_=ot
                            )

    nc.compile()
    return nc


def _get_runner():
    """Build the bass program once and return a cached jitted SPMD callable."""
    global _RUNNER
    if _RUNNER is not None:
        return _RUNNER
    _RUNNER = _make_runner()
    return _RUNNER


def _make_runner():
    """Uncached: build the bass program and a jitted SPMD callable.

    Mirrors concourse.bass2jax.run_bass_via_pjrt's multi-core path, but
    returns the jitted function so repeat calls skip retracing.
    """
    import concourse.mybir as mybir
    import jax
    from concourse.bass2jax import (
        _bass_exec_p,
        install_neuronx_cc_hook,
        partition_id_tensor,
    )
    from jax.experimental.shard_map import shard_map
    from jax.sharding import Mesh, PartitionSpec

    _install_neff_cache()
    nc = _build_nc()
    install_neuronx_cc_hook()

    partition_name = nc.partition_id_tensor.name if nc.partition_id_tensor else None
    in_names = []
    out_names = []
    out_avals = []
    out_shapes = []
    for alloc in nc.m.functions[0].allocations:
        if not isinstance(alloc, mybir.MemoryLocationSet):
            continue
        name = alloc.memorylocations[0].name
        if alloc.kind == "ExternalInput":
            if name == partition_name:
                continue
            in_names.append(name)
        elif alloc.kind == "ExternalOutput":
            out_names.append(name)
            shape = tuple(alloc.tensor_shape)
            dtype = mybir.dt.np(alloc.dtype)
            out_avals.append(jax.core.ShapedArray(shape, dtype))
            out_shapes.append((shape, dtype))
    n_params = len(in_names)
    n_outs = len(out_names)
    all_in_names = in_names + out_names
    if partition_name is not None:
        all_in_names = all_in_names + [partition_name]

    def _body(*args):
        operands = list(args)
        if partition_name is not None:
            operands.append(partition_id_tensor())
        outs = _bass_exec_p.bind(
            *operands,
            out_avals=tuple(out_avals),
            in_names=tuple(all_in_names),
            out_names=tuple(out_names),
            lowering_input_output_aliases=(),
            sim_require_finite=True,
            sim_require_nnan=True,
            nc=nc,
        )
        return tuple(outs)

    devices = jax.devices()[:N_CORES]
    assert len(devices) == N_CORES, f"need {N_CORES} devices, got {len(devices)}"
    mesh = Mesh(np.asarray(devices), ("core",))
    in_specs = (PartitionSpec("core"),) * (n_params + n_outs)
    out_specs = (PartitionSpec("core"),) * n_outs
    donate = tuple(range(n_params, n_params + n_outs))
    sharded = jax.jit(
        shard_map(
            _body, mesh=mesh, in_specs=in_specs, out_specs=out_specs, check_rep=False
        ),
        donate_argnums=donate,
        keep_unused=True,
    )

    global _CHAIN_PARTS
    _CHAIN_PARTS = (nc, in_names, out_names, tuple(out_avals), partition_name, mesh)
    return (sharded, in_names, out_names, out_shapes)


_CHAIN_PARTS = None


def make_chain(k):
    """Return a jitted callable(x, wt, b, z) -> z' that executes the bass
    NEFF k times back-to-back inside one dispatch, chained through the
    output buffer (true data dependency). For timing: slope over k isolates
    per-execution time from the fixed axon dispatch overhead."""
    import jax
    from concourse.bass2jax import _bass_exec_p, partition_id_tensor
    from jax.experimental.shard_map import shard_map
    from jax.sharding import PartitionSpec

    _get_runner()
    nc, in_names, out_names, out_avals, partition_name, mesh = _CHAIN_PARTS
    all_in_names = list(in_names) + list(out_names)
    if partition_name is not None:
        all_in_names = all_in_names + [partition_name]

    def body_k(*args):
        args = list(args)
        z = args[-1]
        for _ in range(k):
            operands = args[:-1] + [z]
            if partition_name is not None:
                operands.append(partition_id_tensor())
            (z,) = _bass_exec_p.bind(
                *operands,
                out_avals=tuple(out_avals),
                in_names=tuple(all_in_names),
                out_names=tuple(out_names),
                lowering_input_output_aliases=(),
                sim_require_finite=True,
                sim_require_nnan=True,
                nc=nc,
            )
        return z

    n_args = len(in_names) + 1
    return jax.jit(
        shard_map(
            body_k,
            mesh=mesh,
            in_specs=(PartitionSpec("core"),) * n_args,
            out_specs=PartitionSpec("core"),
            check_rep=False,
        ),
        donate_argnums=n_args - 1,
        keep_unused=True,
    )


def prepare_inputs(inp, blocks, bias):
    """Host-side shard prep -> global concat arrays keyed by bass input name.

    Block sharding means the global (axis-0 concat over cores) arrays are:
      x  = inp itself               (2048, 8192)
      wt = per-block transposes     (2048, 256)
      b  = bias as (8*128, 2): per core (128, 2) with [p, m] = bias_c[m*128+p]
    """
    if MM_DTYPE == "bf16":
        import ml_dtypes

        xdt = ml_dtypes.bfloat16
    elif MM_DTYPE == "fp16":
        xdt = np.float16
    else:
        xdt = np.float32
    inp = np.ascontiguousarray(np.asarray(inp, dtype=np.float32).astype(xdt))
    blocks = np.asarray(blocks, dtype=np.float32)
    bias = np.asarray(bias, dtype=np.float32)

    wt = np.ascontiguousarray(blocks.transpose(0, 2, 1).astype(xdt)).reshape(
        NUM_BLOCKS * BLOCK_DIM, BLOCK_DIM
    )
    b = np.ascontiguousarray(
        bias.reshape(NUM_BLOCKS, BLOCK_DIM // P, P).transpose(0, 2, 1)
    ).reshape(NUM_BLOCKS * P, BLOCK_DIM // P)
    return {"x": inp, "wt": wt, "b": b}


def run_prepared(global_ins):
    """Run the SPMD program on globally-concatenated inputs; returns raw
    jax output arrays (caller reassembles/blocks)."""
    import jax.numpy as jnp

    sharded, in_names, out_names, out_shapes = _get_runner()
    args = [global_ins[name] for name in in_names]
    zeros = [
        jnp.zeros((N_CORES * shape[0], *shape[1:]), dtype) for shape, dtype in out_shapes
    ]
    outs = sharded(*args, *zeros)
    return dict(zip(out_names, outs))


def kernel(inp, blocks, bias):
    global_ins = prepare_inputs(inp, blocks, bias)
    outs = run_prepared(global_ins)
    out = np.asarray(outs["out"])  # (8*256, 8192) — block rows already in order
    return out


if __name__ == "__main__":
    rng = np.random.default_rng(0)
    inp = rng.standard_normal((N_ROWS, B_COLS), dtype=np.float32)
    blocks = rng.standard_normal((NUM_BLOCKS, BLOCK_DIM, BLOCK_DIM), dtype=np.float32)
    bias = rng.standard_normal((N_ROWS,), dtype=np.float32)
    out = kernel(inp, blocks, bias)
    x = inp.reshape(NUM_BLOCKS, BLOCK_DIM, -1)
    ref = np.einsum("kij,kjb->kib", blocks, x).reshape(N_ROWS, -1) + bias[:, None]
    err = np.abs(out - ref)
    rel = np.linalg.norm(out - ref) / np.linalg.norm(ref)
    print("max abs err:", err.max(), "rel:", rel)
